# revision 12
# baseline (speedup 1.0000x reference)
"""Trainium2 Bass kernel for nn_CATA_30339648979575 (criss-cross attention x2 +
multi-scale depthwise conv).

Self-contained: builds two SPMD NEFFs (block1+conv, block2) and runs them on 8
NeuronCores via run_bass_kernel_spmd. Host shards batch x row-quarters; inputs
are row-rolled per core so one NEFF serves all quarters.
"""
import os
import numpy as np

import concourse.bass as bass
import concourse.mybir as mybir
import concourse.tile as tile
from concourse.bass_utils import run_bass_kernel_spmd
from concourse.masks import make_identity

F32 = mybir.dt.float32
F32R = mybir.dt.float32r


def _r(ap):
    return ap.bitcast(F32R)

B, N, C = 2, 16384, 512
H = W = 128
CQ = C // 8  # 64
QROWS = 32           # image rows per core quarter
QHW = QROWS * W      # 4096
HALO = 38            # QROWS + 6 conv halo rows
PADW = W + 6         # 134, zero-padded conv width

# ---------------------------------------------------------------------------
# walrus workaround: split TileContext exit-drain waits across single-wait nops
# ---------------------------------------------------------------------------
_patched = False


def _apply_drain_patch():
    global _patched
    if _patched:
        return
    _patched = True
    tile_mod = tile

    def _drain_and_barrier_split(self, tick_clock, wait_clock):
        nc = self.nc
        nop = nc.sync.nop(nofuse=True, hint="drain_waits")
        wait_clock.add_sem_waits(
            nop.ins, tile_mod.ScopedClock({None: tick_clock.global_clock})
        )
        waits = list(nop.ins.sync_info.on_wait)
        if len(waits) > 1:
            nop.ins.sync_info.on_wait = waits[:1]
            for w in waits[1:]:
                n2 = nc.sync.nop(nofuse=True, hint="drain_waits")
                if n2.ins.sync_info is None:
                    n2.ins.sync_info = mybir.SyncInfo(on_wait=[w], on_update=[])
                else:
                    n2.ins.sync_info.on_wait = [w]
        nc.sync.drain()

        nc.all_engine_barrier()
        assert self.sems is not None
        popped = nc._tile_sem_poison_stack.pop()
        assert popped is self._sem_poison
        nc.clear_and_free_semaphores(list(self.sems.allocated().values()))
        nc.all_engine_barrier()

    tile_mod.TileContext._drain_and_barrier = _drain_and_barrier_split


_ws_counter = [0]


def _split_waits(nc):
    """Walrus in this env allows at most ONE sync wait per instruction.
    Hoist extra waits onto same-engine nops inserted before the instruction."""
    for fn in nc.m.functions:
        for blk in fn.blocks:
            insts = list(blk.instructions)
            out = []
            changed = False
            for inst in insts:
                si = inst.sync_info
                waits = list(si.on_wait) if si is not None and si.on_wait else []
                if len(waits) > 1:
                    changed = True
                    for w in waits[:-1]:
                        _ws_counter[0] += 1
                        nop = mybir.InstNoOp(
                            name=f"WS-{_ws_counter[0]}", ins=[], outs=[])
                        nop.engine = inst.engine
                        nop.sync_info = mybir.SyncInfo(on_wait=[w], on_update=[])
                        out.append(nop)
                    si.on_wait = waits[-1:]
                out.append(inst)
            if changed:
                blk.instructions = out


def _bcast_ap(ap_1d, count):
    """Partition-broadcast AP: repeat a DRAM row `count` times across partitions."""
    return bass.AP(tensor=ap_1d.tensor, offset=ap_1d.offset,
                   ap=[[0, count]] + list(ap_1d.ap))


# ---------------------------------------------------------------------------
# NEFF builder
# ---------------------------------------------------------------------------


def build_block(with_conv: bool) -> bass.Bass:
    _apply_drain_patch()
    nc = bass.Bass()
    name = "cc1" if with_conv else "cc2"
    nc.name = name

    xT = nc.dram_tensor("xT", [C, N], F32, kind="ExternalInput")
    xq = nc.dram_tensor("xq", [QHW, C], F32, kind="ExternalInput")
    wqT = nc.dram_tensor("wqT", [C, CQ], F32, kind="ExternalInput")
    wkT = nc.dram_tensor("wkT", [C, CQ], F32, kind="ExternalInput")
    wvT = nc.dram_tensor("wvT", [C, C], F32, kind="ExternalInput")
    bq = nc.dram_tensor("bq", [1, CQ], F32, kind="ExternalInput")
    bk = nc.dram_tensor("bk", [1, CQ], F32, kind="ExternalInput")
    bv = nc.dram_tensor("bv", [1, C], F32, kind="ExternalInput")
    gamma = nc.dram_tensor("gamma", [1, 1], F32, kind="ExternalInput")
    if with_conv:
        xh = nc.dram_tensor("xh", [C, HALO * W], F32, kind="ExternalInput")
        diag = nc.dram_tensor("diag", [4, 49, 128, 128], F32, kind="ExternalInput")
        bcomb = nc.dram_tensor("bcomb", [1, C], F32, kind="ExternalInput")
    out_q = nc.dram_tensor("out_q", [QHW, C], F32, kind="ExternalOutput")

    # DRAM scratch
    vt_d = nc.dram_tensor("vt_d", [N, C], F32)
    ah_d = nc.dram_tensor("ah_d", [QROWS, W, C], F32)
    ew_d = nc.dram_tensor("ew_d", [H, H, W], F32)
    p_d = nc.dram_tensor("p_d", [QROWS, H, W], F32)
    if with_conv:
        xc_d = nc.dram_tensor("xc_d", [QROWS, W, C], F32)

    with tile.TileContext(nc) as tc:
        with tc.tile_pool(name="const", bufs=1) as cst:
            # constants
            wq_sb = cst.tile([128, 4, CQ], F32)
            nc.sync.dma_start(out=wq_sb, in_=wqT.rearrange("(t p) c -> p t c", p=128))
            wk_sb = cst.tile([128, 4, CQ], F32)
            nc.sync.dma_start(out=wk_sb, in_=wkT.rearrange("(t p) c -> p t c", p=128))
            wv_sb = cst.tile([128, 4, C], F32)
            nc.sync.dma_start(out=wv_sb, in_=wvT.rearrange("(t p) c -> p t c", p=128))
            bq_sb = cst.tile([CQ, 1], F32)
            nc.gpsimd.dma_start(out=bq_sb, in_=bq.rearrange("o c -> c o"))
            bk_sb = cst.tile([CQ, 1], F32)
            nc.gpsimd.dma_start(out=bk_sb, in_=bk.rearrange("o c -> c o"))
            bv_sb = cst.tile([128, C], F32)
            nc.gpsimd.dma_start(out=bv_sb, in_=_bcast_ap(bv[0, :], 128))
            g_sb = cst.tile([128, 1], F32)
            nc.gpsimd.dma_start(out=g_sb, in_=_bcast_ap(gamma[0, :], 128))
            ident = cst.tile([128, 128], F32)
            make_identity(nc, ident)
            if with_conv:
                bc_sb = cst.tile([128, 4], F32)
                nc.gpsimd.dma_start(out=bc_sb, in_=bcomb[0, :].rearrange("(t p) -> p t", p=128))
            mW = cst.tile([H, W], F32)
            sw = cst.tile([H, W], F32)
            rw = cst.tile([H, W], F32)

            # ---------------- phases 1-4 (q/k resident) ----------------
            with tc.tile_pool(name="qkpool", bufs=1) as qkp, \
                 tc.tile_pool(name="p14", bufs=2) as sp:
                q_sb = qkp.tile([CQ, N], F32)
                k_sb = qkp.tile([CQ, N], F32)
                q3 = q_sb.rearrange("p (i j) -> p i j", j=W)
                k3 = k_sb.rearrange("p (i j) -> p i j", j=W)

                # phase 1: projections q,k,v
                ppcm = tc.tile_pool(name="ps1", bufs=2, space="PSUM")
                pp = ppcm.__enter__()
                for blk in range(32):
                    xt_t = []
                    for ct in range(4):
                        t = sp.tile([128, 512], F32, name=f"xt{ct}", tag=f"xt{ct}")
                        nc.sync.dma_start(
                            out=t, in_=xT[ct * 128:(ct + 1) * 128,
                                          blk * 512:(blk + 1) * 512])
                        xt_t.append(t)
                    q_ps = pp.tile([CQ, 512], F32, name="q_ps", tag="q_ps")
                    k_ps = pp.tile([CQ, 512], F32, name="k_ps", tag="k_ps")
                    for ct in range(4):
                        nc.tensor.matmul(q_ps, lhsT=_r(wq_sb[:, ct, :]),
                                         rhs=_r(xt_t[ct][:, :]),
                                         start=(ct == 0), stop=(ct == 3))
                    for ct in range(4):
                        nc.tensor.matmul(k_ps, lhsT=_r(wk_sb[:, ct, :]),
                                         rhs=_r(xt_t[ct][:, :]),
                                         start=(ct == 0), stop=(ct == 3))
                    nc.scalar.activation(out=q_sb[:, blk * 512:(blk + 1) * 512],
                                         in_=q_ps,
                                         func=mybir.ActivationFunctionType.Identity,
                                         bias=bq_sb, scale=1.0)
                    nc.scalar.activation(out=k_sb[:, blk * 512:(blk + 1) * 512],
                                         in_=k_ps,
                                         func=mybir.ActivationFunctionType.Identity,
                                         bias=bk_sb, scale=1.0)
                    for sub in range(4):
                        v_ps = pp.tile([128, C], F32, name="v_ps", tag="v_ps")
                        for ct in range(4):
                            nc.tensor.matmul(
                                v_ps,
                                lhsT=_r(xt_t[ct][:, sub * 128:(sub + 1) * 128]),
                                rhs=_r(wv_sb[:, ct, :]),
                                start=(ct == 0), stop=(ct == 3))
                        vt_sb = sp.tile([128, C], F32, name="vt_sb", tag="vt_sb", bufs=3)
                        nc.vector.tensor_tensor(out=vt_sb, in0=v_ps, in1=bv_sb,
                                                op=mybir.AluOpType.add)
                        hw0 = blk * 512 + sub * 128
                        nc.scalar.dma_start(out=vt_d[hw0:hw0 + 128, :], in_=vt_sb)

                ppcm.__exit__(None, None, None)
                # phase 2: H branch (per column j)
                vt3 = vt_d.rearrange("(i j) c -> i j c", j=W)
                ppcm = tc.tile_pool(name="ps2", bufs=2, space="PSUM")
                pp = ppcm.__enter__()
                for j in range(W):
                    fh_ps = pp.tile([H, H], F32, name="fh_ps", tag="fh_ps")
                    nc.tensor.matmul(fh_ps, lhsT=_r(k3[:, :, j]),
                                     rhs=_r(q3[:, :, j]),
                                     start=True, stop=True)
                    negm = sp.tile([H, 1], F32, name="negm", tag="negm", bufs=3)
                    nc.vector.tensor_reduce(out=negm, in_=fh_ps,
                                            axis=mybir.AxisListType.X,
                                            op=mybir.AluOpType.max, negate=True)
                    aht = sp.tile([H, H], F32, name="aht", tag="aht", bufs=3)
                    ssum = sp.tile([H, 1], F32, name="ssum", tag="ssum", bufs=3)
                    nc.scalar.activation(out=aht, in_=fh_ps,
                                         func=mybir.ActivationFunctionType.Exp,
                                         bias=negm, scale=1.0, accum_out=ssum)
                    rsum = sp.tile([H, 1], F32, name="rsum", tag="rsum", bufs=3)
                    nc.vector.reciprocal(out=rsum, in_=ssum)
                    nc.vector.tensor_scalar_mul(out=aht, in0=aht, scalar1=rsum)
                    vt_j = sp.tile([H, C], F32, name="vt_j", tag="vt_j", bufs=3)
                    nc.sync.dma_start(out=vt_j, in_=vt3[:, j, :])
                    oh_ps = pp.tile([QROWS, C], F32, name="oh_ps", tag="oh_ps")
                    nc.tensor.matmul(oh_ps, lhsT=_r(aht[:, 0:QROWS]),
                                     rhs=_r(vt_j[:, :]),
                                     start=True, stop=True)
                    oh_sb = sp.tile([QROWS, C], F32, name="oh_sb", tag="oh_sb", bufs=3)
                    nc.vector.tensor_copy(out=oh_sb, in_=oh_ps)
                    nc.scalar.dma_start(out=ah_d[:, j, :], in_=oh_sb)

                ppcm.__exit__(None, None, None)
                # phase 3: W pass A (energies + running max)
                ppcm = tc.tile_pool(name="ps3", bufs=2, space="PSUM")
                pp = ppcm.__enter__()
                for i in range(H):
                    ew_ps = pp.tile([H, W], F32, name="ew_ps", tag="ew_ps", bufs=3)
                    nc.tensor.matmul(ew_ps, lhsT=_r(k3[:, i, :]),
                                     rhs=_r(q3[:, i, :]),
                                     start=True, stop=True)
                    if i == 0:
                        nc.vector.tensor_copy(out=mW, in_=ew_ps)
                    else:
                        nc.vector.tensor_tensor(out=mW, in0=mW, in1=ew_ps,
                                                op=mybir.AluOpType.max)
                    ew_sb = sp.tile([H, W], F32, name="ew_sb", tag="ew_sb", bufs=3)
                    nc.vector.tensor_copy(out=ew_sb, in_=ew_ps)
                    nc.scalar.dma_start(out=ew_d[i, :, :], in_=ew_sb)

                ppcm.__exit__(None, None, None)
                # phase 4: W pass B (P = exp(E - mW), SW = sum_i P)
                nc.vector.memset(sw, 0.0)
                for i in range(H):
                    e_sb = sp.tile([H, W], F32, name="e_sb", tag="e_sb", bufs=3)
                    nc.sync.dma_start(out=e_sb, in_=ew_d[i, :, :])
                    p_sb = sp.tile([H, W], F32, name="p_sb", tag="p_sb", bufs=3)
                    nc.vector.tensor_tensor(out=p_sb, in0=e_sb, in1=mW,
                                            op=mybir.AluOpType.subtract)
                    nc.scalar.activation(out=p_sb, in_=p_sb,
                                         func=mybir.ActivationFunctionType.Exp)
                    nc.vector.tensor_tensor(out=sw, in0=sw, in1=p_sb,
                                            op=mybir.AluOpType.add)
                    if i < QROWS:
                        nc.sync.dma_start(out=p_d[i, :, :], in_=p_sb)
                nc.vector.reciprocal(out=rw, in_=sw)

            # ---------------- phase 5: conv (q/k freed) ----------------
            if with_conv:
                xh3 = xh.rearrange("c (r j) -> c r j", j=W)
                with tc.tile_pool(name="convp", bufs=1) as cp, \
                     tc.tile_pool(name="convs", bufs=2) as cs, \
                     tc.tile_pool(name="convps", bufs=2, space="PSUM") as cps_pool:
                    conv_ct = []
                    for ct in range(4):
                        c_sb = cp.tile([128, QROWS * PADW], F32, name=f"conv{ct}")
                        conv_ct.append(c_sb)
                    for ct in range(4):
                        fcm = cs.tile([128, HALO * PADW + 16], F32, name="fcm", tag="fcm")
                        nc.vector.memset(fcm, 0.0)
                        fcm3 = fcm[:, 8:8 + HALO * PADW].rearrange("p (r j) -> p r j", j=PADW)
                        nc.sync.dma_start(out=fcm3[:, :, 3:3 + W],
                                          in_=xh3[ct * 128:(ct + 1) * 128, :, :])
                        dg = cs.tile([128, 49, 128], F32, name="dg", tag="dg")
                        nc.sync.dma_start(
                            out=dg, in_=diag[ct].rearrange("t p m -> p t m"))
                        flat = QROWS * PADW  # 4288
                        for o in range(0, flat, 512):
                            csz = min(512, flat - o)
                            cps = cps_pool.tile([128, 512], F32, name="cps", tag="cps")
                            for t in range(49):
                                dy, dx = t // 7 - 3, t % 7 - 3
                                in_off = 8 + o + (3 + dy) * PADW + dx
                                nc.tensor.matmul(
                                    cps[:, 0:csz],
                                    lhsT=_r(dg[:, t, :]),
                                    rhs=_r(fcm[:, in_off:in_off + csz]),
                                    start=(t == 0), stop=(t == 48))
                            nc.scalar.activation(
                                out=conv_ct[ct][:, o:o + csz], in_=cps[:, 0:csz],
                                func=mybir.ActivationFunctionType.Identity,
                                bias=bc_sb[:, ct:ct + 1], scale=1.0)
                    # transpose to spatial rows and add xq
                    for il in range(QROWS):
                        tp_ps = cps_pool.tile([128, C], F32, name="tp_ps", tag="tp_ps")
                        for ct in range(4):
                            nc.tensor.transpose(
                                tp_ps[:, ct * 128:(ct + 1) * 128],
                                conv_ct[ct][:, il * PADW + 3: il * PADW + 3 + W],
                                ident)
                        xq_t = cs.tile([128, C], F32, name="xq_t", tag="xq_t", bufs=3)
                        nc.sync.dma_start(out=xq_t,
                                          in_=xq[il * W:(il + 1) * W, :])
                        xc_sb = cs.tile([128, C], F32, name="xc_sb", tag="xc_sb", bufs=3)
                        nc.vector.tensor_tensor(out=xc_sb, in0=tp_ps, in1=xq_t,
                                                op=mybir.AluOpType.add)
                        nc.sync.dma_start(out=xc_d[il, :, :], in_=xc_sb)

            # ---------------- phase 6: W pass C + finishing ----------------
            with tc.tile_pool(name="fin", bufs=3) as fp, \
                 tc.tile_pool(name="finps", bufs=2, space="PSUM") as fpp:
                for il in range(QROWS):
                    p_t = fp.tile([H, W], F32, name="p_t", tag="p_t")
                    nc.sync.dma_start(out=p_t, in_=p_d[il, :, :])
                    awt = fp.tile([H, W], F32, name="awt", tag="awt")
                    nc.vector.tensor_tensor(out=awt, in0=p_t, in1=rw,
                                            op=mybir.AluOpType.mult)
                    vt_rb = fp.tile([H, C], F32, name="vt_rb", tag="vt_rb")
                    nc.sync.dma_start(out=vt_rb,
                                      in_=vt_d[il * W:(il + 1) * W, :])
                    ow_ps = fpp.tile([W, C], F32, name="ow_ps", tag="ow_ps")
                    nc.tensor.matmul(ow_ps, lhsT=_r(awt[:, :]), rhs=_r(vt_rb[:, :]),
                                     start=True, stop=True)
                    ah_t = fp.tile([W, C], F32, name="ah_t", tag="ah_t")
                    nc.sync.dma_start(out=ah_t, in_=ah_d[il, :, :])
                    s1 = fp.tile([W, C], F32, name="s1", tag="s1")
                    nc.vector.tensor_tensor(out=s1, in0=ow_ps, in1=ah_t,
                                            op=mybir.AluOpType.add)
                    xc_t = fp.tile([W, C], F32, name="xc_t", tag="xc_t")
                    if with_conv:
                        nc.sync.dma_start(out=xc_t, in_=xc_d[il, :, :])
                    else:
                        nc.sync.dma_start(out=xc_t,
                                          in_=xq[il * W:(il + 1) * W, :])
                    o_t = fp.tile([W, C], F32, name="o_t", tag="o_t")
                    nc.vector.scalar_tensor_tensor(out=o_t, in0=s1, scalar=g_sb,
                                                   in1=xc_t,
                                                   op0=mybir.AluOpType.mult,
                                                   op1=mybir.AluOpType.add)
                    nc.sync.dma_start(out=out_q[il * W:(il + 1) * W, :], in_=o_t)
    return nc


# ---------------------------------------------------------------------------
# host-side prep + run
# ---------------------------------------------------------------------------


def _prep_core(x_b, qidx, with_halo):
    """Per-core rolled inputs for one batch sample x_b [N, C]."""
    feat3 = x_b.reshape(H, W, C)
    perm = [(r + QROWS * qidx) % H for r in range(H)]
    rolled = feat3[perm].reshape(N, C)
    xT = np.ascontiguousarray(rolled.T)
    xq = np.ascontiguousarray(x_b[qidx * QHW:(qidx + 1) * QHW])
    out = {"xT": xT, "xq": xq}
    if with_halo:
        slab = np.zeros((HALO, W, C), np.float32)
        for r in range(HALO):
            src = qidx * QROWS - 3 + r
            if 0 <= src < H:
                slab[r] = feat3[src]
        out["xh"] = np.ascontiguousarray(slab.reshape(HALO * W, C).T)
    return out


_cache = {}
last_results = []


def _get_nc(with_conv):
    key = bool(with_conv)
    if key not in _cache:
        nc = build_block(with_conv)
        _split_waits(nc)
        for f in nc.m.functions:
            for blk in f.blocks:
                pass
        _cache[key] = nc
    return _cache[key]


def _run_block(x_full, wq, bq, wk, bk, wv, bv, gamma, conv=None):
    """x_full: [B, N, C]. conv: None or (diag, bcomb, wcomb-unused). Returns [B, N, C]."""
    with_conv = conv is not None
    nc = _get_nc(with_conv)
    base = {
        "wqT": np.ascontiguousarray(wq.T), "bq": bq.reshape(1, CQ),
        "wkT": np.ascontiguousarray(wk.T), "bk": bk.reshape(1, CQ),
        "wvT": np.ascontiguousarray(wv.T), "bv": bv.reshape(1, C),
        "gamma": np.asarray(gamma, np.float32).reshape(1, 1),
    }
    if with_conv:
        diag, bcomb = conv
        base["diag"] = diag
        base["bcomb"] = bcomb.reshape(1, C)
    in_maps = []
    for core in range(8):
        b, qidx = core // 4, core % 4
        m = dict(base)
        m.update(_prep_core(x_full[b], qidx, with_conv))
        in_maps.append(m)
    trace = os.environ.get("CC_TRACE", "") == "1"
    res = run_bass_kernel_spmd(nc, in_maps, core_ids=list(range(8)),
                               trace=trace,
                               trace_cores=[0] if trace else None)
    last_results.append(res)
    out = np.empty((B, N, C), np.float32)
    for core in range(8):
        b, qidx = core // 4, core % 4
        out[b, qidx * QHW:(qidx + 1) * QHW] = res.results[core]["out_q"]
    return out


def kernel(**inputs):
    x = np.asarray(inputs["x"], np.float32)
    wcomb = np.array(inputs["wp7"][:, 0], np.float32)
    wcomb[:, 1:6, 1:6] += np.asarray(inputs["wp5"][:, 0])
    wcomb[:, 2:5, 2:5] += np.asarray(inputs["wp3"][:, 0])
    bcomb = np.asarray(inputs["bp7"] + inputs["bp5"] + inputs["bp3"], np.float32)
    diag = np.zeros((4, 49, 128, 128), np.float32)
    idx = np.arange(128)
    for ct in range(4):
        for t in range(49):
            diag[ct, t, idx, idx] = wcomb[ct * 128:(ct + 1) * 128, t // 7, t % 7]

    out_a = _run_block(x, inputs["wq"], inputs["bq"], inputs["wk"], inputs["bk"],
                       inputs["wv"], inputs["bv"], inputs["gamma"],
                       conv=(diag, bcomb))
    out1 = _run_block(out_a, inputs["wq1"], inputs["bq1"], inputs["wk1"],
                      inputs["bk1"], inputs["wv1"], inputs["bv1"], inputs["gamma1"])
    return out1


# revision 14
# speedup vs baseline: 1.0762x; 1.0762x over previous
"""Trainium2 Bass kernel for nn_CATA_30339648979575 (criss-cross attention x2 +
multi-scale depthwise conv).

Self-contained: builds two SPMD NEFFs (block1+conv, block2) and runs them on 8
NeuronCores via run_bass_kernel_spmd. Host shards batch x row-quarters; inputs
are row-rolled per core so one NEFF serves all quarters.
"""
import os
import numpy as np

import concourse.bass as bass
import concourse.mybir as mybir
import concourse.tile as tile
from concourse.bass_utils import run_bass_kernel_spmd
from concourse.masks import make_identity

F32 = mybir.dt.float32
F32R = mybir.dt.float32r


def _r(ap):
    return ap.bitcast(F32R)

B, N, C = 2, 16384, 512
H = W = 128
CQ = C // 8  # 64
QROWS = 32           # image rows per core quarter
QHW = QROWS * W      # 4096
HALO = 38            # QROWS + 6 conv halo rows
PADW = W + 6         # 134, zero-padded conv width

# ---------------------------------------------------------------------------
# walrus workaround: split TileContext exit-drain waits across single-wait nops
# ---------------------------------------------------------------------------
_patched = False


def _apply_drain_patch():
    global _patched
    if _patched:
        return
    _patched = True
    tile_mod = tile

    def _drain_and_barrier_split(self, tick_clock, wait_clock):
        nc = self.nc
        nop = nc.sync.nop(nofuse=True, hint="drain_waits")
        wait_clock.add_sem_waits(
            nop.ins, tile_mod.ScopedClock({None: tick_clock.global_clock})
        )
        waits = list(nop.ins.sync_info.on_wait)
        if len(waits) > 1:
            nop.ins.sync_info.on_wait = waits[:1]
            for w in waits[1:]:
                n2 = nc.sync.nop(nofuse=True, hint="drain_waits")
                if n2.ins.sync_info is None:
                    n2.ins.sync_info = mybir.SyncInfo(on_wait=[w], on_update=[])
                else:
                    n2.ins.sync_info.on_wait = [w]
        nc.sync.drain()

        nc.all_engine_barrier()
        assert self.sems is not None
        popped = nc._tile_sem_poison_stack.pop()
        assert popped is self._sem_poison
        nc.clear_and_free_semaphores(list(self.sems.allocated().values()))
        nc.all_engine_barrier()

    tile_mod.TileContext._drain_and_barrier = _drain_and_barrier_split


_ws_counter = [0]


def _split_waits(nc):
    """Walrus in this env allows at most ONE sync wait per instruction.
    Hoist extra waits onto same-engine nops inserted before the instruction."""
    for fn in nc.m.functions:
        for blk in fn.blocks:
            insts = list(blk.instructions)
            out = []
            changed = False
            for inst in insts:
                si = inst.sync_info
                waits = list(si.on_wait) if si is not None and si.on_wait else []
                if len(waits) > 1:
                    changed = True
                    for w in waits[:-1]:
                        _ws_counter[0] += 1
                        nop = mybir.InstNoOp(
                            name=f"WS-{_ws_counter[0]}", ins=[], outs=[])
                        nop.engine = inst.engine
                        nop.sync_info = mybir.SyncInfo(on_wait=[w], on_update=[])
                        out.append(nop)
                    si.on_wait = waits[-1:]
                out.append(inst)
            if changed:
                blk.instructions = out


def _bcast_ap(ap_1d, count):
    """Partition-broadcast AP: repeat a DRAM row `count` times across partitions."""
    return bass.AP(tensor=ap_1d.tensor, offset=ap_1d.offset,
                   ap=[[0, count]] + list(ap_1d.ap))


# ---------------------------------------------------------------------------
# NEFF builder
# ---------------------------------------------------------------------------


def build_block(with_conv: bool) -> bass.Bass:
    _apply_drain_patch()
    nc = bass.Bass()
    name = "cc1" if with_conv else "cc2"
    nc.name = name

    xT = nc.dram_tensor("xT", [C, N], F32, kind="ExternalInput")
    xq = nc.dram_tensor("xq", [QHW, C], F32, kind="ExternalInput")
    wqT = nc.dram_tensor("wqT", [C, CQ], F32, kind="ExternalInput")
    wkT = nc.dram_tensor("wkT", [C, CQ], F32, kind="ExternalInput")
    wvT = nc.dram_tensor("wvT", [C, C], F32, kind="ExternalInput")
    bq = nc.dram_tensor("bq", [1, CQ], F32, kind="ExternalInput")
    bk = nc.dram_tensor("bk", [1, CQ], F32, kind="ExternalInput")
    bv = nc.dram_tensor("bv", [1, C], F32, kind="ExternalInput")
    gamma = nc.dram_tensor("gamma", [1, 1], F32, kind="ExternalInput")
    if with_conv:
        xh = nc.dram_tensor("xh", [C, HALO * W], F32, kind="ExternalInput")
        diag = nc.dram_tensor("diag", [4, 49, 128, 128], F32, kind="ExternalInput")
        bcomb = nc.dram_tensor("bcomb", [1, C], F32, kind="ExternalInput")
    out_q = nc.dram_tensor("out_q", [QHW, C], F32, kind="ExternalOutput")

    # DRAM scratch
    vt_d = nc.dram_tensor("vt_d", [N, C], F32)
    ah_d = nc.dram_tensor("ah_d", [QROWS, W, C], F32)
    ew_d = nc.dram_tensor("ew_d", [H, H, W], F32)
    p_d = nc.dram_tensor("p_d", [QROWS, H, W], F32)
    if with_conv:
        xc_d = nc.dram_tensor("xc_d", [QROWS, W, C], F32)

    with tile.TileContext(nc) as tc:
        with tc.tile_pool(name="const", bufs=1) as cst:
            # constants
            wq_sb = cst.tile([128, 4, CQ], F32)
            nc.sync.dma_start(out=wq_sb, in_=wqT.rearrange("(t p) c -> p t c", p=128))
            wk_sb = cst.tile([128, 4, CQ], F32)
            nc.sync.dma_start(out=wk_sb, in_=wkT.rearrange("(t p) c -> p t c", p=128))
            wv_sb = cst.tile([128, 4, C], F32)
            nc.sync.dma_start(out=wv_sb, in_=wvT.rearrange("(t p) c -> p t c", p=128))
            bq_sb = cst.tile([CQ, 1], F32)
            nc.gpsimd.dma_start(out=bq_sb, in_=bq.rearrange("o c -> c o"))
            bk_sb = cst.tile([CQ, 1], F32)
            nc.gpsimd.dma_start(out=bk_sb, in_=bk.rearrange("o c -> c o"))
            bv_sb = cst.tile([128, C], F32)
            nc.gpsimd.dma_start(out=bv_sb, in_=_bcast_ap(bv[0, :], 128))
            g_sb = cst.tile([128, 1], F32)
            nc.gpsimd.dma_start(out=g_sb, in_=_bcast_ap(gamma[0, :], 128))
            ident = cst.tile([128, 128], F32)
            make_identity(nc, ident)
            if with_conv:
                bc_sb = cst.tile([128, 4], F32)
                nc.gpsimd.dma_start(out=bc_sb, in_=bcomb[0, :].rearrange("(t p) -> p t", p=128))
            mW = cst.tile([H, W], F32)
            sw = cst.tile([H, W], F32)
            rw = cst.tile([H, W], F32)

            # ---------------- phases 1-4 (q/k resident) ----------------
            with tc.tile_pool(name="qkpool", bufs=1) as qkp:
                q_sb = qkp.tile([CQ, N], F32)
                k_sb = qkp.tile([CQ, N], F32)
                q3 = q_sb.rearrange("p (i j) -> p i j", j=W)
                k3 = k_sb.rearrange("p (i j) -> p i j", j=W)

                # phase 1: projections q,k,v
                xTr = xT.rearrange("(t p) n -> p t n", p=128)
                ppcm = tc.tile_pool(name="ps1", bufs=2, space="PSUM")
                pp = ppcm.__enter__()
                spcm = tc.tile_pool(name="sp1", bufs=2)
                sp = spcm.__enter__()
                for blk in range(32):
                    xt_t = sp.tile([128, 4, 512], F32, name="xt_t", tag="xt_t",
                                   bufs=3)
                    nc.sync.dma_start(
                        out=xt_t, in_=xTr[:, :, blk * 512:(blk + 1) * 512])
                    q_ps = pp.tile([CQ, 512], F32, name="q_ps", tag="q_ps")
                    k_ps = pp.tile([CQ, 512], F32, name="k_ps", tag="k_ps")
                    for ct in range(4):
                        nc.tensor.matmul(q_ps, lhsT=_r(wq_sb[:, ct, :]),
                                         rhs=_r(xt_t[:, ct, :]),
                                         start=(ct == 0), stop=(ct == 3))
                    for ct in range(4):
                        nc.tensor.matmul(k_ps, lhsT=_r(wk_sb[:, ct, :]),
                                         rhs=_r(xt_t[:, ct, :]),
                                         start=(ct == 0), stop=(ct == 3))
                    nc.scalar.activation(out=q_sb[:, blk * 512:(blk + 1) * 512],
                                         in_=q_ps,
                                         func=mybir.ActivationFunctionType.Identity,
                                         bias=bq_sb, scale=1.0)
                    nc.scalar.activation(out=k_sb[:, blk * 512:(blk + 1) * 512],
                                         in_=k_ps,
                                         func=mybir.ActivationFunctionType.Identity,
                                         bias=bk_sb, scale=1.0)
                    vt_sb = sp.tile([128, 4, C], F32, name="vt_sb", tag="vt_sb",
                                    bufs=3)
                    for sub in range(4):
                        v_ps = pp.tile([128, C], F32, name="v_ps", tag="v_ps")
                        for ct in range(4):
                            nc.tensor.matmul(
                                v_ps,
                                lhsT=_r(xt_t[:, ct, sub * 128:(sub + 1) * 128]),
                                rhs=_r(wv_sb[:, ct, :]),
                                start=(ct == 0), stop=(ct == 3))
                        nc.vector.tensor_tensor(out=vt_sb[:, sub, :], in0=v_ps,
                                                in1=bv_sb,
                                                op=mybir.AluOpType.add)
                    nc.gpsimd.dma_start(
                        out=vt_d[blk * 512:(blk + 1) * 512, :].rearrange(
                            "(s p) c -> p s c", p=128),
                        in_=vt_sb)

                ppcm.__exit__(None, None, None)
                spcm.__exit__(None, None, None)
                # phase 2: H branch (columns in groups of 4)
                vt3 = vt_d.rearrange("(i j) c -> i j c", j=W)
                ppcm = tc.tile_pool(name="ps2", bufs=2, space="PSUM")
                pp = ppcm.__enter__()
                spcm = tc.tile_pool(name="sp2", bufs=2)
                sp = spcm.__enter__()
                for j0 in range(0, W, 4):
                    vt_j = sp.tile([H, 4, C], F32, name="vt_j", tag="vt_j", bufs=2)
                    nc.sync.dma_start(out=vt_j, in_=vt3[:, j0:j0 + 4, :])
                    oh_sb = sp.tile([QROWS, 4, C], F32, name="oh_sb", tag="oh_sb",
                                    bufs=2)
                    for dj in range(4):
                        j = j0 + dj
                        fh_ps = pp.tile([H, H], F32, name="fh_ps", tag="fh_ps")
                        nc.tensor.matmul(fh_ps, lhsT=_r(k3[:, :, j]),
                                         rhs=_r(q3[:, :, j]),
                                         start=True, stop=True)
                        negm = sp.tile([H, 1], F32, name="negm", tag="negm", bufs=3)
                        nc.vector.tensor_reduce(out=negm, in_=fh_ps,
                                                axis=mybir.AxisListType.X,
                                                op=mybir.AluOpType.max, negate=True)
                        aht = sp.tile([H, H], F32, name="aht", tag="aht", bufs=3)
                        ssum = sp.tile([H, 1], F32, name="ssum", tag="ssum", bufs=3)
                        nc.scalar.activation(out=aht, in_=fh_ps,
                                             func=mybir.ActivationFunctionType.Exp,
                                             bias=negm, scale=1.0, accum_out=ssum)
                        rsum = sp.tile([H, 1], F32, name="rsum", tag="rsum", bufs=3)
                        nc.vector.reciprocal(out=rsum, in_=ssum)
                        nc.vector.tensor_scalar_mul(out=aht[:, 0:QROWS],
                                                    in0=aht[:, 0:QROWS],
                                                    scalar1=rsum)
                        oh_ps = pp.tile([QROWS, C], F32, name="oh_ps", tag="oh_ps")
                        nc.tensor.matmul(oh_ps, lhsT=_r(aht[:, 0:QROWS]),
                                         rhs=_r(vt_j[:, dj, :]),
                                         start=True, stop=True)
                        nc.vector.tensor_copy(out=oh_sb[:, dj, :], in_=oh_ps)
                    nc.gpsimd.dma_start(out=ah_d[:, j0:j0 + 4, :], in_=oh_sb)

                ppcm.__exit__(None, None, None)
                spcm.__exit__(None, None, None)
                # phase 3: W pass A (energies + running max), rows in groups of 4
                ppcm = tc.tile_pool(name="ps3", bufs=2, space="PSUM")
                pp = ppcm.__enter__()
                spcm = tc.tile_pool(name="sp3", bufs=2)
                sp = spcm.__enter__()
                for i0 in range(0, H, 4):
                    ew_sb = sp.tile([H, 4, W], F32, name="ew_sb", tag="ew_sb",
                                    bufs=2)
                    for di in range(4):
                        i = i0 + di
                        ew_ps = pp.tile([H, W], F32, name="ew_ps", tag="ew_ps",
                                        bufs=3)
                        nc.tensor.matmul(ew_ps, lhsT=_r(k3[:, i, :]),
                                         rhs=_r(q3[:, i, :]),
                                         start=True, stop=True)
                        if i == 0:
                            nc.vector.tensor_copy(out=mW, in_=ew_ps)
                        else:
                            nc.vector.tensor_tensor(out=mW, in0=mW, in1=ew_ps,
                                                    op=mybir.AluOpType.max)
                        nc.vector.tensor_copy(out=ew_sb[:, di, :], in_=ew_ps)
                    nc.gpsimd.dma_start(
                        out=ew_d[i0:i0 + 4, :, :].rearrange("i k j -> k i j"),
                        in_=ew_sb)

                ppcm.__exit__(None, None, None)
                spcm.__exit__(None, None, None)
                # phase 4: W pass B (P = exp(E - mW), SW = sum_i P)
                spcm = tc.tile_pool(name="sp4", bufs=2)
                sp = spcm.__enter__()
                nc.vector.memset(sw, 0.0)
                mW4 = bass.AP(tensor=mW.tensor, offset=mW.offset,
                              ap=[list(mW.ap[0]), [0, 4]] + [list(mW.ap[1])])
                for i0 in range(0, H, 4):
                    e_sb = sp.tile([H, 4, W], F32, name="e_sb", tag="e_sb", bufs=3)
                    nc.sync.dma_start(
                        out=e_sb, in_=ew_d[i0:i0 + 4, :, :].rearrange("i k j -> k i j"))
                    p_sb = sp.tile([H, 4, W], F32, name="p_sb", tag="p_sb", bufs=3)
                    nc.vector.tensor_tensor(out=p_sb, in0=e_sb, in1=mW4,
                                            op=mybir.AluOpType.subtract)
                    nc.scalar.activation(out=p_sb, in_=p_sb,
                                         func=mybir.ActivationFunctionType.Exp)
                    for di in range(4):
                        nc.vector.tensor_tensor(out=sw, in0=sw, in1=p_sb[:, di, :],
                                                op=mybir.AluOpType.add)
                    if i0 < QROWS:
                        nc.gpsimd.dma_start(
                            out=p_d[i0:i0 + 4, :, :].rearrange("i k j -> k i j"),
                            in_=p_sb)
                nc.vector.reciprocal(out=rw, in_=sw)
                spcm.__exit__(None, None, None)

            # ---------------- phase 5: conv (q/k freed) ----------------
            if with_conv:
                xh3 = xh.rearrange("c (r j) -> c r j", j=W)
                with tc.tile_pool(name="convp", bufs=1) as cp, \
                     tc.tile_pool(name="convs", bufs=2) as cs, \
                     tc.tile_pool(name="convps", bufs=2, space="PSUM") as cps_pool:
                    conv_ct = []
                    for ct in range(4):
                        c_sb = cp.tile([128, QROWS * PADW], F32, name=f"conv{ct}")
                        conv_ct.append(c_sb)
                    for ct in range(4):
                        fcm = cs.tile([128, HALO * PADW + 16], F32, name="fcm", tag="fcm")
                        nc.vector.memset(fcm, 0.0)
                        fcm3 = fcm[:, 8:8 + HALO * PADW].rearrange("p (r j) -> p r j", j=PADW)
                        nc.sync.dma_start(out=fcm3[:, :, 3:3 + W],
                                          in_=xh3[ct * 128:(ct + 1) * 128, :, :])
                        dg = cs.tile([128, 49, 128], F32, name="dg", tag="dg")
                        nc.sync.dma_start(
                            out=dg, in_=diag[ct].rearrange("t p m -> p t m"))
                        flat = QROWS * PADW  # 4288
                        for o in range(0, flat, 512):
                            csz = min(512, flat - o)
                            cps = cps_pool.tile([128, 512], F32, name="cps", tag="cps")
                            for t in range(49):
                                dy, dx = t // 7 - 3, t % 7 - 3
                                in_off = 8 + o + (3 + dy) * PADW + dx
                                nc.tensor.matmul(
                                    cps[:, 0:csz],
                                    lhsT=_r(dg[:, t, :]),
                                    rhs=_r(fcm[:, in_off:in_off + csz]),
                                    start=(t == 0), stop=(t == 48))
                            nc.scalar.activation(
                                out=conv_ct[ct][:, o:o + csz], in_=cps[:, 0:csz],
                                func=mybir.ActivationFunctionType.Identity,
                                bias=bc_sb[:, ct:ct + 1], scale=1.0)
                    # transpose to spatial rows and add xq
                    for il in range(QROWS):
                        tp_ps = cps_pool.tile([128, C], F32, name="tp_ps", tag="tp_ps")
                        for ct in range(4):
                            nc.tensor.transpose(
                                tp_ps[:, ct * 128:(ct + 1) * 128],
                                conv_ct[ct][:, il * PADW + 3: il * PADW + 3 + W],
                                ident)
                        xq_t = cs.tile([128, C], F32, name="xq_t", tag="xq_t", bufs=3)
                        nc.sync.dma_start(out=xq_t,
                                          in_=xq[il * W:(il + 1) * W, :])
                        xc_sb = cs.tile([128, C], F32, name="xc_sb", tag="xc_sb", bufs=3)
                        nc.vector.tensor_tensor(out=xc_sb, in0=tp_ps, in1=xq_t,
                                                op=mybir.AluOpType.add)
                        nc.sync.dma_start(out=xc_d[il, :, :], in_=xc_sb)

            # ---------------- phase 6: W pass C + finishing ----------------
            with tc.tile_pool(name="fin", bufs=3) as fp, \
                 tc.tile_pool(name="finps", bufs=2, space="PSUM") as fpp:
                for il in range(QROWS):
                    p_t = fp.tile([H, W], F32, name="p_t", tag="p_t")
                    nc.sync.dma_start(out=p_t, in_=p_d[il, :, :])
                    awt = fp.tile([H, W], F32, name="awt", tag="awt")
                    nc.vector.tensor_tensor(out=awt, in0=p_t, in1=rw,
                                            op=mybir.AluOpType.mult)
                    vt_rb = fp.tile([H, C], F32, name="vt_rb", tag="vt_rb")
                    nc.sync.dma_start(out=vt_rb,
                                      in_=vt_d[il * W:(il + 1) * W, :])
                    ow_ps = fpp.tile([W, C], F32, name="ow_ps", tag="ow_ps")
                    nc.tensor.matmul(ow_ps, lhsT=_r(awt[:, :]), rhs=_r(vt_rb[:, :]),
                                     start=True, stop=True)
                    ah_t = fp.tile([W, C], F32, name="ah_t", tag="ah_t")
                    nc.sync.dma_start(out=ah_t, in_=ah_d[il, :, :])
                    s1 = fp.tile([W, C], F32, name="s1", tag="s1")
                    nc.vector.tensor_tensor(out=s1, in0=ow_ps, in1=ah_t,
                                            op=mybir.AluOpType.add)
                    xc_t = fp.tile([W, C], F32, name="xc_t", tag="xc_t")
                    if with_conv:
                        nc.sync.dma_start(out=xc_t, in_=xc_d[il, :, :])
                    else:
                        nc.sync.dma_start(out=xc_t,
                                          in_=xq[il * W:(il + 1) * W, :])
                    o_t = fp.tile([W, C], F32, name="o_t", tag="o_t")
                    nc.vector.scalar_tensor_tensor(out=o_t, in0=s1, scalar=g_sb,
                                                   in1=xc_t,
                                                   op0=mybir.AluOpType.mult,
                                                   op1=mybir.AluOpType.add)
                    nc.sync.dma_start(out=out_q[il * W:(il + 1) * W, :], in_=o_t)
    return nc


# ---------------------------------------------------------------------------
# host-side prep + run
# ---------------------------------------------------------------------------


def _prep_core(x_b, qidx, with_halo):
    """Per-core rolled inputs for one batch sample x_b [N, C]."""
    feat3 = x_b.reshape(H, W, C)
    perm = [(r + QROWS * qidx) % H for r in range(H)]
    rolled = feat3[perm].reshape(N, C)
    xT = np.ascontiguousarray(rolled.T)
    xq = np.ascontiguousarray(x_b[qidx * QHW:(qidx + 1) * QHW])
    out = {"xT": xT, "xq": xq}
    if with_halo:
        slab = np.zeros((HALO, W, C), np.float32)
        for r in range(HALO):
            src = qidx * QROWS - 3 + r
            if 0 <= src < H:
                slab[r] = feat3[src]
        out["xh"] = np.ascontiguousarray(slab.reshape(HALO * W, C).T)
    return out


_cache = {}
last_results = []


def _get_nc(with_conv):
    key = bool(with_conv)
    if key not in _cache:
        nc = build_block(with_conv)
        _split_waits(nc)
        for f in nc.m.functions:
            for blk in f.blocks:
                pass
        _cache[key] = nc
    return _cache[key]


def _run_block(x_full, wq, bq, wk, bk, wv, bv, gamma, conv=None):
    """x_full: [B, N, C]. conv: None or (diag, bcomb, wcomb-unused). Returns [B, N, C]."""
    with_conv = conv is not None
    nc = _get_nc(with_conv)
    base = {
        "wqT": np.ascontiguousarray(wq.T), "bq": bq.reshape(1, CQ),
        "wkT": np.ascontiguousarray(wk.T), "bk": bk.reshape(1, CQ),
        "wvT": np.ascontiguousarray(wv.T), "bv": bv.reshape(1, C),
        "gamma": np.asarray(gamma, np.float32).reshape(1, 1),
    }
    if with_conv:
        diag, bcomb = conv
        base["diag"] = diag
        base["bcomb"] = bcomb.reshape(1, C)
    in_maps = []
    for core in range(8):
        b, qidx = core // 4, core % 4
        m = dict(base)
        m.update(_prep_core(x_full[b], qidx, with_conv))
        in_maps.append(m)
    trace = os.environ.get("CC_TRACE", "") == "1"
    res = run_bass_kernel_spmd(nc, in_maps, core_ids=list(range(8)),
                               trace=trace,
                               trace_cores=[0] if trace else None)
    last_results.append(res)
    out = np.empty((B, N, C), np.float32)
    for core in range(8):
        b, qidx = core // 4, core % 4
        out[b, qidx * QHW:(qidx + 1) * QHW] = res.results[core]["out_q"]
    return out


def kernel(**inputs):
    x = np.asarray(inputs["x"], np.float32)
    wcomb = np.array(inputs["wp7"][:, 0], np.float32)
    wcomb[:, 1:6, 1:6] += np.asarray(inputs["wp5"][:, 0])
    wcomb[:, 2:5, 2:5] += np.asarray(inputs["wp3"][:, 0])
    bcomb = np.asarray(inputs["bp7"] + inputs["bp5"] + inputs["bp3"], np.float32)
    diag = np.zeros((4, 49, 128, 128), np.float32)
    idx = np.arange(128)
    for ct in range(4):
        for t in range(49):
            diag[ct, t, idx, idx] = wcomb[ct * 128:(ct + 1) * 128, t // 7, t % 7]

    out_a = _run_block(x, inputs["wq"], inputs["bq"], inputs["wk"], inputs["bk"],
                       inputs["wv"], inputs["bv"], inputs["gamma"],
                       conv=(diag, bcomb))
    out1 = _run_block(out_a, inputs["wq1"], inputs["bq1"], inputs["wk1"],
                      inputs["bk1"], inputs["wv1"], inputs["bv1"], inputs["gamma1"])
    return out1


# revision 15
# speedup vs baseline: 1.1530x; 1.0714x over previous
"""Trainium2 Bass kernel for nn_CATA_30339648979575 (criss-cross attention x2 +
multi-scale depthwise conv).

Self-contained: builds two SPMD NEFFs (block1+conv, block2) and runs them on 8
NeuronCores via run_bass_kernel_spmd. Host shards batch x row-quarters; inputs
are row-rolled per core so one NEFF serves all quarters.
"""
import os
import numpy as np

import concourse.bass as bass
import concourse.mybir as mybir
import concourse.tile as tile
from concourse.bass_utils import run_bass_kernel_spmd
from concourse.masks import make_identity

F32 = mybir.dt.float32
F32R = mybir.dt.float32r


def _r(ap):
    return ap.bitcast(F32R)

B, N, C = 2, 16384, 512
H = W = 128
CQ = C // 8  # 64
QROWS = 32           # image rows per core quarter
QHW = QROWS * W      # 4096
HALO = 38            # QROWS + 6 conv halo rows
PADW = W + 6         # 134, zero-padded conv width

# ---------------------------------------------------------------------------
# walrus workaround: split TileContext exit-drain waits across single-wait nops
# ---------------------------------------------------------------------------
_patched = False


def _apply_drain_patch():
    global _patched
    if _patched:
        return
    _patched = True
    tile_mod = tile

    def _drain_and_barrier_split(self, tick_clock, wait_clock):
        nc = self.nc
        nop = nc.sync.nop(nofuse=True, hint="drain_waits")
        wait_clock.add_sem_waits(
            nop.ins, tile_mod.ScopedClock({None: tick_clock.global_clock})
        )
        waits = list(nop.ins.sync_info.on_wait)
        if len(waits) > 1:
            nop.ins.sync_info.on_wait = waits[:1]
            for w in waits[1:]:
                n2 = nc.sync.nop(nofuse=True, hint="drain_waits")
                if n2.ins.sync_info is None:
                    n2.ins.sync_info = mybir.SyncInfo(on_wait=[w], on_update=[])
                else:
                    n2.ins.sync_info.on_wait = [w]
        nc.sync.drain()

        nc.all_engine_barrier()
        assert self.sems is not None
        popped = nc._tile_sem_poison_stack.pop()
        assert popped is self._sem_poison
        nc.clear_and_free_semaphores(list(self.sems.allocated().values()))
        nc.all_engine_barrier()

    tile_mod.TileContext._drain_and_barrier = _drain_and_barrier_split


_ws_counter = [0]


def _split_waits(nc):
    """Walrus in this env allows at most ONE sync wait per instruction.
    Hoist extra waits onto same-engine nops inserted before the instruction."""
    for fn in nc.m.functions:
        for blk in fn.blocks:
            insts = list(blk.instructions)
            out = []
            changed = False
            for inst in insts:
                si = inst.sync_info
                waits = list(si.on_wait) if si is not None and si.on_wait else []
                if len(waits) > 1:
                    changed = True
                    for w in waits[:-1]:
                        _ws_counter[0] += 1
                        nop = mybir.InstNoOp(
                            name=f"WS-{_ws_counter[0]}", ins=[], outs=[])
                        nop.engine = inst.engine
                        nop.sync_info = mybir.SyncInfo(on_wait=[w], on_update=[])
                        out.append(nop)
                    si.on_wait = waits[-1:]
                out.append(inst)
            if changed:
                blk.instructions = out


def _bcast_ap(ap_1d, count):
    """Partition-broadcast AP: repeat a DRAM row `count` times across partitions."""
    return bass.AP(tensor=ap_1d.tensor, offset=ap_1d.offset,
                   ap=[[0, count]] + list(ap_1d.ap))


# ---------------------------------------------------------------------------
# NEFF builder
# ---------------------------------------------------------------------------


def build_block(with_conv: bool) -> bass.Bass:
    _apply_drain_patch()
    nc = bass.Bass()
    name = "cc1" if with_conv else "cc2"
    nc.name = name

    xT = nc.dram_tensor("xT", [C, N], F32, kind="ExternalInput")
    xq = nc.dram_tensor("xq", [QHW, C], F32, kind="ExternalInput")
    wqT = nc.dram_tensor("wqT", [C, CQ], F32, kind="ExternalInput")
    wkT = nc.dram_tensor("wkT", [C, CQ], F32, kind="ExternalInput")
    wvT = nc.dram_tensor("wvT", [C, C], F32, kind="ExternalInput")
    bq = nc.dram_tensor("bq", [1, CQ], F32, kind="ExternalInput")
    bk = nc.dram_tensor("bk", [1, CQ], F32, kind="ExternalInput")
    bv = nc.dram_tensor("bv", [1, C], F32, kind="ExternalInput")
    gamma = nc.dram_tensor("gamma", [1, 1], F32, kind="ExternalInput")
    if with_conv:
        xh = nc.dram_tensor("xh", [C, HALO * W], F32, kind="ExternalInput")
        diag = nc.dram_tensor("diag", [4, 49, 128, 128], F32, kind="ExternalInput")
        bcomb = nc.dram_tensor("bcomb", [1, C], F32, kind="ExternalInput")
    out_q = nc.dram_tensor("out_q", [QHW, C], F32, kind="ExternalOutput")

    # DRAM scratch
    vt_d = nc.dram_tensor("vt_d", [N, C], F32)
    ah_d = nc.dram_tensor("ah_d", [QROWS, W, C], F32)
    ew_d = nc.dram_tensor("ew_d", [H, H, W], F32)
    p_d = nc.dram_tensor("p_d", [QROWS, H, W], F32)
    if with_conv:
        xc_d = nc.dram_tensor("xc_d", [QROWS, W, C], F32)

    with tile.TileContext(nc) as tc:
        with tc.tile_pool(name="const", bufs=1) as cst:
            # constants
            wq_sb = cst.tile([128, 4, CQ], F32)
            nc.sync.dma_start(out=wq_sb, in_=wqT.rearrange("(t p) c -> p t c", p=128))
            wk_sb = cst.tile([128, 4, CQ], F32)
            nc.sync.dma_start(out=wk_sb, in_=wkT.rearrange("(t p) c -> p t c", p=128))
            wv_sb = cst.tile([128, 4, C], F32)
            nc.sync.dma_start(out=wv_sb, in_=wvT.rearrange("(t p) c -> p t c", p=128))
            bq_sb = cst.tile([CQ, 1], F32)
            nc.gpsimd.dma_start(out=bq_sb, in_=bq.rearrange("o c -> c o"))
            bk_sb = cst.tile([CQ, 1], F32)
            nc.gpsimd.dma_start(out=bk_sb, in_=bk.rearrange("o c -> c o"))
            bv_sb = cst.tile([128, C], F32)
            nc.gpsimd.dma_start(out=bv_sb, in_=_bcast_ap(bv[0, :], 128))
            g_sb = cst.tile([128, 1], F32)
            nc.gpsimd.dma_start(out=g_sb, in_=_bcast_ap(gamma[0, :], 128))
            ident = cst.tile([128, 128], F32)
            make_identity(nc, ident)
            if with_conv:
                bc_sb = cst.tile([128, 4], F32)
                nc.gpsimd.dma_start(out=bc_sb, in_=bcomb[0, :].rearrange("(t p) -> p t", p=128))
            mW = cst.tile([H, W], F32)
            sw = cst.tile([H, W], F32)
            rw = cst.tile([H, W], F32)

            # ---------------- phases 1-4 (q/k resident) ----------------
            with tc.tile_pool(name="qkpool", bufs=1) as qkp:
                q_sb = qkp.tile([CQ, N], F32)
                k_sb = qkp.tile([CQ, N], F32)
                q3 = q_sb.rearrange("p (i j) -> p i j", j=W)
                k3 = k_sb.rearrange("p (i j) -> p i j", j=W)

                # phase 1: projections q,k,v
                xTr = xT.rearrange("(t p) n -> p t n", p=128)
                ppcm = tc.tile_pool(name="ps1", bufs=2, space="PSUM")
                pp = ppcm.__enter__()
                spcm = tc.tile_pool(name="sp1", bufs=2)
                sp = spcm.__enter__()
                for blk in range(32):
                    xt_t = sp.tile([128, 4, 512], F32, name="xt_t", tag="xt_t",
                                   bufs=3)
                    nc.sync.dma_start(
                        out=xt_t, in_=xTr[:, :, blk * 512:(blk + 1) * 512])
                    q_ps = pp.tile([CQ, 512], F32, name="q_ps", tag="q_ps")
                    k_ps = pp.tile([CQ, 512], F32, name="k_ps", tag="k_ps")
                    for ct in range(4):
                        nc.tensor.matmul(q_ps, lhsT=_r(wq_sb[:, ct, :]),
                                         rhs=_r(xt_t[:, ct, :]),
                                         start=(ct == 0), stop=(ct == 3))
                    for ct in range(4):
                        nc.tensor.matmul(k_ps, lhsT=_r(wk_sb[:, ct, :]),
                                         rhs=_r(xt_t[:, ct, :]),
                                         start=(ct == 0), stop=(ct == 3))
                    nc.scalar.activation(out=q_sb[:, blk * 512:(blk + 1) * 512],
                                         in_=q_ps,
                                         func=mybir.ActivationFunctionType.Identity,
                                         bias=bq_sb, scale=1.0)
                    nc.scalar.activation(out=k_sb[:, blk * 512:(blk + 1) * 512],
                                         in_=k_ps,
                                         func=mybir.ActivationFunctionType.Identity,
                                         bias=bk_sb, scale=1.0)
                    vt_sb = sp.tile([128, 4, C], F32, name="vt_sb", tag="vt_sb",
                                    bufs=3)
                    for sub in range(4):
                        v_ps = pp.tile([128, C], F32, name="v_ps", tag="v_ps")
                        for ct in range(4):
                            nc.tensor.matmul(
                                v_ps,
                                lhsT=_r(xt_t[:, ct, sub * 128:(sub + 1) * 128]),
                                rhs=_r(wv_sb[:, ct, :]),
                                start=(ct == 0), stop=(ct == 3))
                        nc.vector.tensor_tensor(out=vt_sb[:, sub, :], in0=v_ps,
                                                in1=bv_sb,
                                                op=mybir.AluOpType.add)
                    nc.gpsimd.dma_start(
                        out=vt_d[blk * 512:(blk + 1) * 512, :].rearrange(
                            "(s p) c -> p s c", p=128),
                        in_=vt_sb)

                ppcm.__exit__(None, None, None)
                spcm.__exit__(None, None, None)
                # phase 2: H branch (columns in groups of 4)
                vt3 = vt_d.rearrange("(i j) c -> i j c", j=W)
                ppcm = tc.tile_pool(name="ps2", bufs=2, space="PSUM")
                pp = ppcm.__enter__()
                spcm = tc.tile_pool(name="sp2", bufs=2)
                sp = spcm.__enter__()
                for j0 in range(0, W, 4):
                    vt_j = sp.tile([H, 4, C], F32, name="vt_j", tag="vt_j", bufs=2)
                    nc.sync.dma_start(out=vt_j, in_=vt3[:, j0:j0 + 4, :])
                    oh_sb = sp.tile([QROWS, 4, C], F32, name="oh_sb", tag="oh_sb",
                                    bufs=2)
                    for dj in range(4):
                        j = j0 + dj
                        fh_ps = pp.tile([H, H], F32, name="fh_ps", tag="fh_ps")
                        nc.tensor.matmul(fh_ps, lhsT=_r(k3[:, :, j]),
                                         rhs=_r(q3[:, :, j]),
                                         start=True, stop=True)
                        negm = sp.tile([H, 1], F32, name="negm", tag="negm", bufs=3)
                        nc.vector.tensor_reduce(out=negm, in_=fh_ps,
                                                axis=mybir.AxisListType.X,
                                                op=mybir.AluOpType.max, negate=True)
                        aht = sp.tile([H, H], F32, name="aht", tag="aht", bufs=3)
                        ssum = sp.tile([H, 1], F32, name="ssum", tag="ssum", bufs=3)
                        nc.scalar.activation(out=aht, in_=fh_ps,
                                             func=mybir.ActivationFunctionType.Exp,
                                             bias=negm, scale=1.0, accum_out=ssum)
                        rsum = sp.tile([H, 1], F32, name="rsum", tag="rsum", bufs=3)
                        nc.vector.reciprocal(out=rsum, in_=ssum)
                        nc.vector.tensor_scalar_mul(out=aht[:, 0:QROWS],
                                                    in0=aht[:, 0:QROWS],
                                                    scalar1=rsum)
                        oh_ps = pp.tile([QROWS, C], F32, name="oh_ps", tag="oh_ps")
                        nc.tensor.matmul(oh_ps, lhsT=_r(aht[:, 0:QROWS]),
                                         rhs=_r(vt_j[:, dj, :]),
                                         start=True, stop=True)
                        nc.scalar.copy(out=oh_sb[:, dj, :], in_=oh_ps)
                    nc.gpsimd.dma_start(out=ah_d[:, j0:j0 + 4, :], in_=oh_sb)

                ppcm.__exit__(None, None, None)
                spcm.__exit__(None, None, None)
                # phase 3: W pass A (energies + running max), rows in groups of 4
                ppcm = tc.tile_pool(name="ps3", bufs=2, space="PSUM")
                pp = ppcm.__enter__()
                spcm = tc.tile_pool(name="sp3", bufs=2)
                sp = spcm.__enter__()
                for i0 in range(0, H, 4):
                    ew_sb = sp.tile([H, 4, W], F32, name="ew_sb", tag="ew_sb",
                                    bufs=2)
                    for di in range(4):
                        i = i0 + di
                        ew_ps = pp.tile([H, W], F32, name="ew_ps", tag="ew_ps",
                                        bufs=3)
                        nc.tensor.matmul(ew_ps, lhsT=_r(k3[:, i, :]),
                                         rhs=_r(q3[:, i, :]),
                                         start=True, stop=True)
                        if i == 0:
                            nc.vector.tensor_copy(out=mW, in_=ew_ps)
                        else:
                            nc.vector.tensor_tensor(out=mW, in0=mW, in1=ew_ps,
                                                    op=mybir.AluOpType.max)
                        nc.vector.tensor_copy(out=ew_sb[:, di, :], in_=ew_ps)
                    nc.gpsimd.dma_start(
                        out=ew_d[i0:i0 + 4, :, :].rearrange("i k j -> k i j"),
                        in_=ew_sb)

                ppcm.__exit__(None, None, None)
                spcm.__exit__(None, None, None)

            # phase 4 (no q/k needed) — runs concurrent with conv below
            with tc.tile_pool(name="sp4", bufs=2) as sp:
                nc.vector.memset(sw, 0.0)
                mW4 = bass.AP(tensor=mW.tensor, offset=mW.offset,
                              ap=[list(mW.ap[0]), [0, 4]] + [list(mW.ap[1])])
                for i0 in range(0, H, 4):
                    e_sb = sp.tile([H, 4, W], F32, name="e_sb", tag="e_sb", bufs=3)
                    nc.sync.dma_start(
                        out=e_sb, in_=ew_d[i0:i0 + 4, :, :].rearrange("i k j -> k i j"))
                    p_sb = sp.tile([H, 4, W], F32, name="p_sb", tag="p_sb", bufs=3)
                    nc.vector.tensor_tensor(out=p_sb, in0=e_sb, in1=mW4,
                                            op=mybir.AluOpType.subtract)
                    nc.scalar.activation(out=p_sb, in_=p_sb,
                                         func=mybir.ActivationFunctionType.Exp)
                    for di in range(4):
                        nc.vector.tensor_tensor(out=sw, in0=sw, in1=p_sb[:, di, :],
                                                op=mybir.AluOpType.add)
                    if i0 < QROWS:
                        nc.gpsimd.dma_start(
                            out=p_d[i0:i0 + 4, :, :].rearrange("i k j -> k i j"),
                            in_=p_sb)
                nc.vector.reciprocal(out=rw, in_=sw)

            # ---------------- phase 5: conv (q/k freed) ----------------
            if with_conv:
                xh3 = xh.rearrange("c (r j) -> c r j", j=W)
                with tc.tile_pool(name="convp", bufs=1) as cp, \
                     tc.tile_pool(name="convs", bufs=2) as cs, \
                     tc.tile_pool(name="convps", bufs=2, space="PSUM") as cps_pool:
                    conv_ct = []
                    for ct in range(4):
                        c_sb = cp.tile([128, QROWS * PADW], F32, name=f"conv{ct}")
                        conv_ct.append(c_sb)
                    for ct in range(4):
                        fcm = cs.tile([128, HALO * PADW + 16], F32, name="fcm", tag="fcm")
                        nc.vector.memset(fcm, 0.0)
                        fcm3 = fcm[:, 8:8 + HALO * PADW].rearrange("p (r j) -> p r j", j=PADW)
                        nc.sync.dma_start(out=fcm3[:, :, 3:3 + W],
                                          in_=xh3[ct * 128:(ct + 1) * 128, :, :])
                        dg = cs.tile([128, 49, 128], F32, name="dg", tag="dg")
                        nc.sync.dma_start(
                            out=dg, in_=diag[ct].rearrange("t p m -> p t m"))
                        flat = QROWS * PADW  # 4288
                        for o in range(0, flat, 512):
                            csz = min(512, flat - o)
                            cps = cps_pool.tile([128, 512], F32, name="cps", tag="cps")
                            for t in range(49):
                                dy, dx = t // 7 - 3, t % 7 - 3
                                in_off = 8 + o + (3 + dy) * PADW + dx
                                nc.tensor.matmul(
                                    cps[:, 0:csz],
                                    lhsT=_r(dg[:, t, :]),
                                    rhs=_r(fcm[:, in_off:in_off + csz]),
                                    start=(t == 0), stop=(t == 48))
                            nc.scalar.activation(
                                out=conv_ct[ct][:, o:o + csz], in_=cps[:, 0:csz],
                                func=mybir.ActivationFunctionType.Identity,
                                bias=bc_sb[:, ct:ct + 1], scale=1.0)
                    # transpose to spatial rows and add xq
                    for il in range(QROWS):
                        tp_ps = cps_pool.tile([128, C], F32, name="tp_ps", tag="tp_ps")
                        for ct in range(4):
                            nc.tensor.transpose(
                                tp_ps[:, ct * 128:(ct + 1) * 128],
                                conv_ct[ct][:, il * PADW + 3: il * PADW + 3 + W],
                                ident)
                        xq_t = cs.tile([128, C], F32, name="xq_t", tag="xq_t", bufs=3)
                        nc.sync.dma_start(out=xq_t,
                                          in_=xq[il * W:(il + 1) * W, :])
                        xc_sb = cs.tile([128, C], F32, name="xc_sb", tag="xc_sb", bufs=3)
                        nc.vector.tensor_tensor(out=xc_sb, in0=tp_ps, in1=xq_t,
                                                op=mybir.AluOpType.add)
                        nc.sync.dma_start(out=xc_d[il, :, :], in_=xc_sb)

            # ---------------- phase 6: W pass C + finishing (pairs) ----------------
            with tc.tile_pool(name="fin", bufs=3) as fp, \
                 tc.tile_pool(name="finps", bufs=3, space="PSUM") as fpp:
                for il0 in range(0, QROWS, 2):
                    p_t = fp.tile([H, 2, W], F32, name="p_t", tag="p_t")
                    nc.sync.dma_start(
                        out=p_t,
                        in_=p_d[il0:il0 + 2, :, :].rearrange("i k j -> k i j"))
                    vt_rb = fp.tile([H, 2, C], F32, name="vt_rb", tag="vt_rb")
                    nc.sync.dma_start(
                        out=vt_rb,
                        in_=vt_d[il0 * W:(il0 + 2) * W, :].rearrange(
                            "(s p) c -> p s c", p=128))
                    ah_t = fp.tile([W, 2, C], F32, name="ah_t", tag="ah_t")
                    nc.scalar.dma_start(
                        out=ah_t,
                        in_=ah_d[il0:il0 + 2, :, :].rearrange("i p c -> p i c"))
                    xc_t = fp.tile([W, 2, C], F32, name="xc_t", tag="xc_t")
                    if with_conv:
                        nc.scalar.dma_start(
                            out=xc_t,
                            in_=xc_d[il0:il0 + 2, :, :].rearrange("i p c -> p i c"))
                    else:
                        nc.scalar.dma_start(
                            out=xc_t,
                            in_=xq[il0 * W:(il0 + 2) * W, :].rearrange(
                                "(s p) c -> p s c", p=128))
                    o_t = fp.tile([W, 2, C], F32, name="o_t", tag="o_t")
                    for d in range(2):
                        awt = fp.tile([H, W], F32, name="awt", tag="awt")
                        nc.vector.tensor_tensor(out=awt, in0=p_t[:, d, :], in1=rw,
                                                op=mybir.AluOpType.mult)
                        ow_ps = fpp.tile([W, C], F32, name="ow_ps", tag="ow_ps")
                        nc.tensor.matmul(ow_ps, lhsT=_r(awt[:, :]),
                                         rhs=_r(vt_rb[:, d, :]),
                                         start=True, stop=True)
                        s1 = fp.tile([W, C], F32, name="s1", tag="s1")
                        nc.vector.tensor_tensor(out=s1, in0=ow_ps,
                                                in1=ah_t[:, d, :],
                                                op=mybir.AluOpType.add)
                        nc.vector.scalar_tensor_tensor(out=o_t[:, d, :], in0=s1,
                                                       scalar=g_sb,
                                                       in1=xc_t[:, d, :],
                                                       op0=mybir.AluOpType.mult,
                                                       op1=mybir.AluOpType.add)
                    nc.gpsimd.dma_start(
                        out=out_q[il0 * W:(il0 + 2) * W, :].rearrange(
                            "(s p) c -> p s c", p=128),
                        in_=o_t)
    return nc


# ---------------------------------------------------------------------------
# host-side prep + run
# ---------------------------------------------------------------------------


def _prep_core(x_b, qidx, with_halo):
    """Per-core rolled inputs for one batch sample x_b [N, C]."""
    feat3 = x_b.reshape(H, W, C)
    perm = [(r + QROWS * qidx) % H for r in range(H)]
    rolled = feat3[perm].reshape(N, C)
    xT = np.ascontiguousarray(rolled.T)
    xq = np.ascontiguousarray(x_b[qidx * QHW:(qidx + 1) * QHW])
    out = {"xT": xT, "xq": xq}
    if with_halo:
        slab = np.zeros((HALO, W, C), np.float32)
        for r in range(HALO):
            src = qidx * QROWS - 3 + r
            if 0 <= src < H:
                slab[r] = feat3[src]
        out["xh"] = np.ascontiguousarray(slab.reshape(HALO * W, C).T)
    return out


_cache = {}
last_results = []


def _get_nc(with_conv):
    key = bool(with_conv)
    if key not in _cache:
        nc = build_block(with_conv)
        _split_waits(nc)
        for f in nc.m.functions:
            for blk in f.blocks:
                pass
        _cache[key] = nc
    return _cache[key]


def _run_block(x_full, wq, bq, wk, bk, wv, bv, gamma, conv=None):
    """x_full: [B, N, C]. conv: None or (diag, bcomb, wcomb-unused). Returns [B, N, C]."""
    with_conv = conv is not None
    nc = _get_nc(with_conv)
    base = {
        "wqT": np.ascontiguousarray(wq.T), "bq": bq.reshape(1, CQ),
        "wkT": np.ascontiguousarray(wk.T), "bk": bk.reshape(1, CQ),
        "wvT": np.ascontiguousarray(wv.T), "bv": bv.reshape(1, C),
        "gamma": np.asarray(gamma, np.float32).reshape(1, 1),
    }
    if with_conv:
        diag, bcomb = conv
        base["diag"] = diag
        base["bcomb"] = bcomb.reshape(1, C)
    in_maps = []
    for core in range(8):
        b, qidx = core // 4, core % 4
        m = dict(base)
        m.update(_prep_core(x_full[b], qidx, with_conv))
        in_maps.append(m)
    trace = os.environ.get("CC_TRACE", "") == "1"
    res = run_bass_kernel_spmd(nc, in_maps, core_ids=list(range(8)),
                               trace=trace,
                               trace_cores=[0] if trace else None)
    last_results.append(res)
    out = np.empty((B, N, C), np.float32)
    for core in range(8):
        b, qidx = core // 4, core % 4
        out[b, qidx * QHW:(qidx + 1) * QHW] = res.results[core]["out_q"]
    return out


def kernel(**inputs):
    x = np.asarray(inputs["x"], np.float32)
    wcomb = np.array(inputs["wp7"][:, 0], np.float32)
    wcomb[:, 1:6, 1:6] += np.asarray(inputs["wp5"][:, 0])
    wcomb[:, 2:5, 2:5] += np.asarray(inputs["wp3"][:, 0])
    bcomb = np.asarray(inputs["bp7"] + inputs["bp5"] + inputs["bp3"], np.float32)
    diag = np.zeros((4, 49, 128, 128), np.float32)
    idx = np.arange(128)
    for ct in range(4):
        for t in range(49):
            diag[ct, t, idx, idx] = wcomb[ct * 128:(ct + 1) * 128, t // 7, t % 7]

    out_a = _run_block(x, inputs["wq"], inputs["bq"], inputs["wk"], inputs["bk"],
                       inputs["wv"], inputs["bv"], inputs["gamma"],
                       conv=(diag, bcomb))
    out1 = _run_block(out_a, inputs["wq1"], inputs["bq1"], inputs["wk1"],
                      inputs["bk1"], inputs["wv1"], inputs["bv1"], inputs["gamma1"])
    return out1


# revision 26
# speedup vs baseline: 2.6361x; 2.2862x over previous
"""Trainium2 Bass kernel for nn_CATA_30339648979575 (criss-cross attention x2 +
multi-scale depthwise conv).

Self-contained: builds two SPMD NEFFs (block1+conv, block2) and runs them on 8
NeuronCores via run_bass_kernel_spmd. Host shards batch x row-quarters; inputs
are row-rolled per core so one NEFF serves all quarters.
"""
import os
import numpy as np
import ml_dtypes

BF16NP = ml_dtypes.bfloat16

import concourse.bass as bass
import concourse.mybir as mybir
import concourse.tile as tile
from concourse.bass_utils import run_bass_kernel_spmd
from concourse.masks import make_identity

F32 = mybir.dt.float32
F32R = mybir.dt.float32r
BF16 = mybir.dt.bfloat16


def _r(ap):
    return ap

B, N, C = 2, 16384, 512
H = W = 128
CQ = C // 8  # 64
QROWS = 32           # image rows per core quarter
QHW = QROWS * W      # 4096
HALO = 38            # QROWS + 6 conv halo rows
PADW = W + 6         # 134, zero-padded conv width

# ---------------------------------------------------------------------------
# walrus workaround: split TileContext exit-drain waits across single-wait nops
# ---------------------------------------------------------------------------
_patched = False


def _apply_drain_patch():
    global _patched
    if _patched:
        return
    _patched = True
    tile_mod = tile

    def _drain_and_barrier_split(self, tick_clock, wait_clock):
        nc = self.nc
        nop = nc.sync.nop(nofuse=True, hint="drain_waits")
        wait_clock.add_sem_waits(
            nop.ins, tile_mod.ScopedClock({None: tick_clock.global_clock})
        )
        waits = list(nop.ins.sync_info.on_wait)
        if len(waits) > 1:
            nop.ins.sync_info.on_wait = waits[:1]
            for w in waits[1:]:
                n2 = nc.sync.nop(nofuse=True, hint="drain_waits")
                if n2.ins.sync_info is None:
                    n2.ins.sync_info = mybir.SyncInfo(on_wait=[w], on_update=[])
                else:
                    n2.ins.sync_info.on_wait = [w]
        nc.sync.drain()

        nc.all_engine_barrier()
        assert self.sems is not None
        popped = nc._tile_sem_poison_stack.pop()
        assert popped is self._sem_poison
        nc.clear_and_free_semaphores(list(self.sems.allocated().values()))
        nc.all_engine_barrier()

    tile_mod.TileContext._drain_and_barrier = _drain_and_barrier_split


_ws_counter = [0]


def _split_waits(nc):
    """Walrus in this env allows at most ONE sync wait per instruction.
    Hoist extra waits onto same-engine nops inserted before the instruction."""
    for fn in nc.m.functions:
        for blk in fn.blocks:
            insts = list(blk.instructions)
            out = []
            changed = False
            for inst in insts:
                si = inst.sync_info
                waits = list(si.on_wait) if si is not None and si.on_wait else []
                if len(waits) > 1:
                    changed = True
                    for w in waits[:-1]:
                        _ws_counter[0] += 1
                        nop = mybir.InstNoOp(
                            name=f"WS-{_ws_counter[0]}", ins=[], outs=[])
                        nop.engine = inst.engine
                        nop.sync_info = mybir.SyncInfo(on_wait=[w], on_update=[])
                        out.append(nop)
                    si.on_wait = waits[-1:]
                out.append(inst)
            if changed:
                blk.instructions = out


def _bcast_ap(ap_1d, count):
    """Partition-broadcast AP: repeat a DRAM row `count` times across partitions."""
    return bass.AP(tensor=ap_1d.tensor, offset=ap_1d.offset,
                   ap=[[0, count]] + list(ap_1d.ap))


# ---------------------------------------------------------------------------
# NEFF builder
# ---------------------------------------------------------------------------


def build_block(with_conv: bool) -> bass.Bass:
    _apply_drain_patch()
    nc = bass.Bass()
    name = "cc1" if with_conv else "cc2"
    nc.name = name

    xT = nc.dram_tensor("xT", [C, N], BF16, kind="ExternalInput")
    xq = nc.dram_tensor("xq", [QHW, C], F32, kind="ExternalInput")
    wqT = nc.dram_tensor("wqT", [C, CQ], BF16, kind="ExternalInput")
    wkT = nc.dram_tensor("wkT", [C, CQ], BF16, kind="ExternalInput")
    wvT = nc.dram_tensor("wvT", [C, C], BF16, kind="ExternalInput")
    bq = nc.dram_tensor("bq", [1, CQ], F32, kind="ExternalInput")
    bk = nc.dram_tensor("bk", [1, CQ], F32, kind="ExternalInput")
    bv = nc.dram_tensor("bv", [1, C], F32, kind="ExternalInput")
    gamma = nc.dram_tensor("gamma", [1, 1], F32, kind="ExternalInput")
    if with_conv:
        xh = nc.dram_tensor("xh", [C, HALO * W], BF16, kind="ExternalInput")
        diag = nc.dram_tensor("diag", [4, 49, 128, 128], BF16, kind="ExternalInput")
        bcomb = nc.dram_tensor("bcomb", [1, C], F32, kind="ExternalInput")
    out_q = nc.dram_tensor("out_q", [QHW, C], F32, kind="ExternalOutput")

    # DRAM scratch
    vt_d = nc.dram_tensor("vt_d", [N, C], BF16)
    ah_d = nc.dram_tensor("ah_d", [QROWS, W, C], BF16)
    ew_d = nc.dram_tensor("ew_d", [H, H, W], F32)
    p_d = nc.dram_tensor("p_d", [QROWS, H, W], F32)
    if with_conv:
        xc_d = nc.dram_tensor("xc_d", [QROWS, W, C], F32)

    with tile.TileContext(nc) as tc:
        with tc.tile_pool(name="const", bufs=1) as cst:
            # constants
            wq_sb = cst.tile([128, 4, CQ], BF16)
            nc.sync.dma_start(out=wq_sb, in_=wqT.rearrange("(t p) c -> p t c", p=128))
            wk_sb = cst.tile([128, 4, CQ], BF16)
            nc.sync.dma_start(out=wk_sb, in_=wkT.rearrange("(t p) c -> p t c", p=128))
            wv_sb = cst.tile([128, 4, C], BF16)
            nc.sync.dma_start(out=wv_sb, in_=wvT.rearrange("(t p) c -> p t c", p=128))
            bq_sb = cst.tile([CQ, 1], F32)
            nc.gpsimd.dma_start(out=bq_sb, in_=bq.rearrange("o c -> c o"))
            bk_sb = cst.tile([CQ, 1], F32)
            nc.gpsimd.dma_start(out=bk_sb, in_=bk.rearrange("o c -> c o"))
            bv_sb = cst.tile([128, C], F32)
            nc.gpsimd.dma_start(out=bv_sb, in_=_bcast_ap(bv[0, :], 128))
            g_sb = cst.tile([128, 1], F32)
            nc.gpsimd.dma_start(out=g_sb, in_=_bcast_ap(gamma[0, :], 128))
            ident = cst.tile([128, 128], BF16)
            make_identity(nc, ident)
            if with_conv:
                bc_sb = cst.tile([128, 4], F32)
                nc.gpsimd.dma_start(out=bc_sb, in_=bcomb[0, :].rearrange("(t p) -> p t", p=128))
            mW = cst.tile([H, W], F32)
            sw = cst.tile([H, W], F32)
            rw = cst.tile([H, W], F32)

            # conv pools opened before qkpool so they outlive it (LIFO)
            if with_conv:
                cpcm = tc.tile_pool(name="convp", bufs=1)
                cp = cpcm.__enter__()
                cscm = tc.tile_pool(name="convs", bufs=2)
                cs = cscm.__enter__()
                cpscm = tc.tile_pool(name="convps", bufs=2, space="PSUM")
                cps_pool = cpscm.__enter__()

            # ---------------- phases 1-4 (q/k resident) ----------------
            with tc.tile_pool(name="qkpool", bufs=1) as qkp:
                q_sb = qkp.tile([CQ, N], BF16)
                k_sb = qkp.tile([CQ, N], BF16)
                q3 = q_sb.rearrange("p (i j) -> p i j", j=W)
                k3 = k_sb.rearrange("p (i j) -> p i j", j=W)

                # phase 1: projections q,k,v
                xTr = xT.rearrange("(t p) n -> p t n", p=128)
                ppcm = tc.tile_pool(name="ps1", bufs=2, space="PSUM")
                pp = ppcm.__enter__()
                spcm = tc.tile_pool(name="sp1", bufs=2)
                sp = spcm.__enter__()
                for blk in range(32):
                    xt_t = sp.tile([128, 4, 512], BF16, name="xt_t", tag="xt_t",
                                   bufs=3)
                    nc.sync.dma_start(
                        out=xt_t, in_=xTr[:, :, blk * 512:(blk + 1) * 512])
                    q_ps = pp.tile([CQ, 512], F32, name="q_ps", tag="q_ps", bufs=1)
                    k_ps = pp.tile([CQ, 512], F32, name="k_ps", tag="k_ps", bufs=1)
                    for ct in range(4):
                        nc.tensor.matmul(q_ps, lhsT=_r(wq_sb[:, ct, :]),
                                         rhs=_r(xt_t[:, ct, :]),
                                         start=(ct == 0), stop=(ct == 3))
                    for ct in range(4):
                        nc.tensor.matmul(k_ps, lhsT=_r(wk_sb[:, ct, :]),
                                         rhs=_r(xt_t[:, ct, :]),
                                         start=(ct == 0), stop=(ct == 3))
                    nc.scalar.activation(out=q_sb[:, blk * 512:(blk + 1) * 512],
                                         in_=q_ps,
                                         func=mybir.ActivationFunctionType.Identity,
                                         bias=bq_sb, scale=1.0)
                    nc.scalar.activation(out=k_sb[:, blk * 512:(blk + 1) * 512],
                                         in_=k_ps,
                                         func=mybir.ActivationFunctionType.Identity,
                                         bias=bk_sb, scale=1.0)
                    vt_sb = sp.tile([128, 4, C], BF16, name="vt_sb", tag="vt_sb",
                                    bufs=3)
                    for sub in range(4):
                        v_ps = pp.tile([128, C], F32, name="v_ps", tag="v_ps")
                        for ct in range(4):
                            nc.tensor.matmul(
                                v_ps,
                                lhsT=_r(xt_t[:, ct, sub * 128:(sub + 1) * 128]),
                                rhs=_r(wv_sb[:, ct, :]),
                                start=(ct == 0), stop=(ct == 3))
                        nc.vector.tensor_tensor(out=vt_sb[:, sub, :], in0=v_ps,
                                                in1=bv_sb,
                                                op=mybir.AluOpType.add)
                    nc.gpsimd.dma_start(
                        out=vt_d[blk * 512:(blk + 1) * 512, :].rearrange(
                            "(s p) c -> p s c", p=128),
                        in_=vt_sb)

                ppcm.__exit__(None, None, None)
                spcm.__exit__(None, None, None)

                # conv setup early so conv matmuls are ready to fill PE gaps
                if with_conv:
                    xh3 = xh.rearrange("c (r j) -> c r j", j=W)
                    conv_ct = []
                    fcm_t = {}
                    dg_t = {}

                    def _conv_prep(ct):
                        fcm = cs.tile([128, HALO * PADW + 16], BF16, name="fcm",
                                      tag="fcm")
                        nc.vector.memset(fcm, 0.0)
                        fcm3 = fcm[:, 8:8 + HALO * PADW].rearrange(
                            "p (r j) -> p r j", j=PADW)
                        nc.sync.dma_start(out=fcm3[:, :, 3:3 + W],
                                          in_=xh3[ct * 128:(ct + 1) * 128, :, :])
                        dg = cs.tile([128, 49, 128], BF16, name="dg", tag="dg")
                        nc.sync.dma_start(
                            out=dg, in_=diag[ct].rearrange("t p m -> p t m"))
                        fcm_t[ct] = fcm
                        dg_t[ct] = dg

                    for ct in range(4):
                        c_sb = cp.tile([128, QROWS * PADW], BF16, name=f"conv{ct}")
                        conv_ct.append(c_sb)
                    for ct in range(2):
                        _conv_prep(ct)

                # phases 2+3 share one scope so the scheduler can interleave
                vt3 = vt_d.rearrange("(i j) c -> i j c", j=W)
                ppcm = tc.tile_pool(name="ps23", bufs=2, space="PSUM")
                pp = ppcm.__enter__()
                spcm = tc.tile_pool(name="sp23", bufs=2)
                sp = spcm.__enter__()
                for j0 in range(0, W, 4):
                    vt_j = sp.tile([H, 4, C], BF16, name="vt_j", tag="vt_j", bufs=2)
                    nc.sync.dma_start(out=vt_j, in_=vt3[:, j0:j0 + 4, :])
                    oh_sb = sp.tile([QROWS, 4, C], BF16, name="oh_sb", tag="oh_sb",
                                    bufs=2)
                    for dj in range(4):
                        j = j0 + dj
                        fh_ps = pp.tile([H, H], F32, name="fh_ps", tag="fh_ps")
                        nc.tensor.matmul(fh_ps, lhsT=_r(k3[:, :, j]),
                                         rhs=_r(q3[:, :, j]),
                                         start=True, stop=True)
                        negm = sp.tile([H, 1], F32, name="negm", tag="negm", bufs=3)
                        nc.vector.tensor_reduce(out=negm, in_=fh_ps,
                                                axis=mybir.AxisListType.X,
                                                op=mybir.AluOpType.max, negate=True)
                        aht = sp.tile([H, H], BF16, name="aht", tag="aht", bufs=3)
                        ssum = sp.tile([H, 1], F32, name="ssum", tag="ssum", bufs=3)
                        nc.scalar.activation(out=aht, in_=fh_ps,
                                             func=mybir.ActivationFunctionType.Exp,
                                             bias=negm, scale=1.0, accum_out=ssum)
                        rsum = sp.tile([H, 1], F32, name="rsum", tag="rsum", bufs=3)
                        nc.vector.reciprocal(out=rsum, in_=ssum)
                        nc.vector.tensor_scalar_mul(out=aht[:, 0:QROWS],
                                                    in0=aht[:, 0:QROWS],
                                                    scalar1=rsum)
                        oh_ps = pp.tile([QROWS, C], F32, name="oh_ps", tag="oh_ps", bufs=2)
                        nc.tensor.matmul(oh_ps, lhsT=_r(aht[:, 0:QROWS]),
                                         rhs=_r(vt_j[:, dj, :]),
                                         start=True, stop=True)
                        nc.scalar.copy(out=oh_sb[:, dj, :], in_=oh_ps)
                    nc.gpsimd.dma_start(out=ah_d[:, j0:j0 + 4, :], in_=oh_sb)

                # phase 3: W pass A (energies + running max), rows in groups of 4
                for i0 in range(0, H, 4):
                    ew_sb = sp.tile([H, 4, W], F32, name="ew_sb", tag="ew_sb",
                                    bufs=2)
                    for di in range(4):
                        i = i0 + di
                        ew_ps = pp.tile([H, W], F32, name="ew_ps", tag="ew_ps",
                                        bufs=1)
                        nc.tensor.matmul(ew_ps, lhsT=_r(k3[:, i, :]),
                                         rhs=_r(q3[:, i, :]),
                                         start=True, stop=True)
                        if i == 0:
                            nc.vector.tensor_copy(out=mW, in_=ew_ps)
                        else:
                            nc.vector.tensor_tensor(out=mW, in0=mW, in1=ew_ps,
                                                    op=mybir.AluOpType.max)
                        nc.vector.tensor_copy(out=ew_sb[:, di, :], in_=ew_ps)
                    nc.gpsimd.dma_start(
                        out=ew_d[i0:i0 + 4, :, :].rearrange("i k j -> k i j"),
                        in_=ew_sb)

                ppcm.__exit__(None, None, None)
                spcm.__exit__(None, None, None)

            # phase 4 — concurrent with conv (pools coexist)
            with tc.tile_pool(name="sp4", bufs=2) as sp:
                nc.vector.memset(sw, 0.0)
                mW4 = bass.AP(tensor=mW.tensor, offset=mW.offset,
                              ap=[list(mW.ap[0]), [0, 4]] + [list(mW.ap[1])])
                for i0 in range(0, H, 4):
                    e_sb = sp.tile([H, 4, W], F32, name="e_sb", tag="e_sb", bufs=3)
                    nc.sync.dma_start(
                        out=e_sb, in_=ew_d[i0:i0 + 4, :, :].rearrange("i k j -> k i j"))
                    p_sb = sp.tile([H, 4, W], F32, name="p_sb", tag="p_sb", bufs=3)
                    nc.vector.tensor_tensor(out=p_sb, in0=e_sb, in1=mW4,
                                            op=mybir.AluOpType.subtract)
                    nc.scalar.activation(out=p_sb, in_=p_sb,
                                         func=mybir.ActivationFunctionType.Exp)
                    for di in range(4):
                        nc.vector.tensor_tensor(out=sw, in0=sw, in1=p_sb[:, di, :],
                                                op=mybir.AluOpType.add)
                    if i0 < QROWS:
                        nc.gpsimd.dma_start(
                            out=p_d[i0:i0 + 4, :, :].rearrange("i k j -> k i j"),
                            in_=p_sb)
                nc.vector.reciprocal(out=rw, in_=sw)

            # ---------------- phase 5: conv (ct-outer, tail row-streamed) ----------------
            if with_conv:
                flat = QROWS * PADW  # 4288
                il_done = 0
                for ct in range(4):
                    if ct >= 2:
                        _conv_prep(ct)
                    fcm = fcm_t[ct]
                    dg = dg_t[ct]
                    for o in range(0, flat, 512):
                        csz = min(512, flat - o)
                        cps = cps_pool.tile([128, 512], F32, name="cps", tag="cps")
                        for t in range(49):
                            dy, dx = t // 7 - 3, t % 7 - 3
                            in_off = 8 + o + (3 + dy) * PADW + dx
                            nc.tensor.matmul(
                                cps[:, 0:csz],
                                lhsT=_r(dg[:, t, :]),
                                rhs=_r(fcm[:, in_off:in_off + csz]),
                                start=(t == 0), stop=(t == 48))
                        nc.scalar.activation(
                            out=conv_ct[ct][:, o:o + csz], in_=cps[:, 0:csz],
                            func=mybir.ActivationFunctionType.Identity,
                            bias=bc_sb[:, ct:ct + 1], scale=1.0)
                        if ct == 3:
                            o_end = o + csz
                            while il_done < QROWS and il_done * PADW + 131 <= o_end:
                                il = il_done
                                tp_ps = cps_pool.tile([128, C], BF16, name="tp_ps",
                                                      tag="tp_ps", bufs=1)
                                for c2 in range(4):
                                    nc.tensor.transpose(
                                        tp_ps[:, c2 * 128:(c2 + 1) * 128],
                                        conv_ct[c2][:, il * PADW + 3:
                                                    il * PADW + 3 + W],
                                        ident)
                                xq_t = cs.tile([128, C], F32, name="xq_t",
                                               tag="xq_t", bufs=3)
                                nc.sync.dma_start(out=xq_t,
                                                  in_=xq[il * W:(il + 1) * W, :])
                                xc_sb = cs.tile([128, C], F32, name="xc_sb",
                                                tag="xc_sb", bufs=3)
                                nc.vector.tensor_tensor(out=xc_sb, in0=tp_ps,
                                                        in1=xq_t,
                                                        op=mybir.AluOpType.add)
                                nc.sync.dma_start(out=xc_d[il, :, :], in_=xc_sb)
                                il_done += 1
                cpscm.__exit__(None, None, None)
                cscm.__exit__(None, None, None)
                cpcm.__exit__(None, None, None)

            # ---------------- phase 6: W pass C + finishing (pairs) ----------------
            with tc.tile_pool(name="fin", bufs=3) as fp, \
                 tc.tile_pool(name="finps", bufs=3, space="PSUM") as fpp:
                for il0 in range(0, QROWS, 2):
                    p_t = fp.tile([H, 2, W], F32, name="p_t", tag="p_t")
                    nc.sync.dma_start(
                        out=p_t,
                        in_=p_d[il0:il0 + 2, :, :].rearrange("i k j -> k i j"))
                    vt_rb = fp.tile([H, 2, C], BF16, name="vt_rb", tag="vt_rb")
                    nc.sync.dma_start(
                        out=vt_rb,
                        in_=vt_d[il0 * W:(il0 + 2) * W, :].rearrange(
                            "(s p) c -> p s c", p=128))
                    ah_t = fp.tile([W, 2, C], BF16, name="ah_t", tag="ah_t")
                    nc.scalar.dma_start(
                        out=ah_t,
                        in_=ah_d[il0:il0 + 2, :, :].rearrange("i p c -> p i c"))
                    xc_t = fp.tile([W, 2, C], F32, name="xc_t", tag="xc_t")
                    if with_conv:
                        nc.scalar.dma_start(
                            out=xc_t,
                            in_=xc_d[il0:il0 + 2, :, :].rearrange("i p c -> p i c"))
                    else:
                        nc.scalar.dma_start(
                            out=xc_t,
                            in_=xq[il0 * W:(il0 + 2) * W, :].rearrange(
                                "(s p) c -> p s c", p=128))
                    o_t = fp.tile([W, 2, C], F32, name="o_t", tag="o_t")
                    for d in range(2):
                        awt = fp.tile([H, W], BF16, name="awt", tag="awt")
                        nc.vector.tensor_tensor(out=awt, in0=p_t[:, d, :], in1=rw,
                                                op=mybir.AluOpType.mult)
                        ow_ps = fpp.tile([W, C], F32, name="ow_ps", tag="ow_ps")
                        nc.tensor.matmul(ow_ps, lhsT=_r(awt[:, :]),
                                         rhs=_r(vt_rb[:, d, :]),
                                         start=True, stop=True)
                        s1 = fp.tile([W, C], F32, name="s1", tag="s1")
                        nc.vector.tensor_tensor(out=s1, in0=ow_ps,
                                                in1=ah_t[:, d, :],
                                                op=mybir.AluOpType.add)
                        nc.vector.scalar_tensor_tensor(out=o_t[:, d, :], in0=s1,
                                                       scalar=g_sb,
                                                       in1=xc_t[:, d, :],
                                                       op0=mybir.AluOpType.mult,
                                                       op1=mybir.AluOpType.add)
                    nc.gpsimd.dma_start(
                        out=out_q[il0 * W:(il0 + 2) * W, :].rearrange(
                            "(s p) c -> p s c", p=128),
                        in_=o_t)
    return nc


# ---------------------------------------------------------------------------
# host-side prep + run
# ---------------------------------------------------------------------------


def _prep_core(x_b, qidx, with_halo):
    """Per-core rolled inputs for one batch sample x_b [N, C]."""
    feat3 = x_b.reshape(H, W, C)
    perm = [(r + QROWS * qidx) % H for r in range(H)]
    rolled = feat3[perm].reshape(N, C)
    xT = np.ascontiguousarray(rolled.T.astype(BF16NP))
    xq = np.ascontiguousarray(x_b[qidx * QHW:(qidx + 1) * QHW])
    out = {"xT": xT, "xq": xq}
    if with_halo:
        slab = np.zeros((HALO, W, C), np.float32)
        for r in range(HALO):
            src = qidx * QROWS - 3 + r
            if 0 <= src < H:
                slab[r] = feat3[src]
        out["xh"] = np.ascontiguousarray(slab.reshape(HALO * W, C).T.astype(BF16NP))
    return out


_cache = {}
last_results = []


def _get_nc(with_conv):
    key = bool(with_conv)
    if key not in _cache:
        nc = build_block(with_conv)
        _split_waits(nc)
        for f in nc.m.functions:
            for blk in f.blocks:
                pass
        _cache[key] = nc
    return _cache[key]


def _run_block(x_full, wq, bq, wk, bk, wv, bv, gamma, conv=None):
    """x_full: [B, N, C]. conv: None or (diag, bcomb, wcomb-unused). Returns [B, N, C]."""
    with_conv = conv is not None
    nc = _get_nc(with_conv)
    base = {
        "wqT": np.ascontiguousarray(wq.T.astype(BF16NP)), "bq": bq.reshape(1, CQ),
        "wkT": np.ascontiguousarray(wk.T.astype(BF16NP)), "bk": bk.reshape(1, CQ),
        "wvT": np.ascontiguousarray(wv.T.astype(BF16NP)), "bv": bv.reshape(1, C),
        "gamma": np.asarray(gamma, np.float32).reshape(1, 1),
    }
    if with_conv:
        diag, bcomb = conv
        base["diag"] = diag.astype(BF16NP)
        base["bcomb"] = bcomb.reshape(1, C)
    in_maps = []
    for core in range(8):
        b, qidx = core // 4, core % 4
        m = dict(base)
        m.update(_prep_core(x_full[b], qidx, with_conv))
        in_maps.append(m)
    trace = os.environ.get("CC_TRACE", "") == "1"
    res = run_bass_kernel_spmd(nc, in_maps, core_ids=list(range(8)),
                               trace=trace,
                               trace_cores=[0] if trace else None)
    last_results.append(res)
    out = np.empty((B, N, C), np.float32)
    for core in range(8):
        b, qidx = core // 4, core % 4
        out[b, qidx * QHW:(qidx + 1) * QHW] = res.results[core]["out_q"]
    return out


def kernel(**inputs):
    x = np.asarray(inputs["x"], np.float32)
    wcomb = np.array(inputs["wp7"][:, 0], np.float32)
    wcomb[:, 1:6, 1:6] += np.asarray(inputs["wp5"][:, 0])
    wcomb[:, 2:5, 2:5] += np.asarray(inputs["wp3"][:, 0])
    bcomb = np.asarray(inputs["bp7"] + inputs["bp5"] + inputs["bp3"], np.float32)
    diag = np.zeros((4, 49, 128, 128), np.float32)
    idx = np.arange(128)
    for ct in range(4):
        for t in range(49):
            diag[ct, t, idx, idx] = wcomb[ct * 128:(ct + 1) * 128, t // 7, t % 7]

    out_a = _run_block(x, inputs["wq"], inputs["bq"], inputs["wk"], inputs["bk"],
                       inputs["wv"], inputs["bv"], inputs["gamma"],
                       conv=(diag, bcomb))
    out1 = _run_block(out_a, inputs["wq1"], inputs["bq1"], inputs["wk1"],
                      inputs["bk1"], inputs["wv1"], inputs["bv1"], inputs["gamma1"])
    return out1
